# revision 95
# baseline (speedup 1.0000x reference)
"""Cayley soliton propagator on 8 Trainium2 NeuronCores.

Math: the Hamiltonian stencil H (jnp.roll-based) is a circulant matrix along D,
so the whole Cayley step (I + i*dt/2*H)^-1 (I - i*dt/2*H) is one complex
circulant matrix M, computed on the host from ham_w via an FFT of the stencil
symbol.  M's kernel decays exponentially; taps beyond +-BAND are dropped
(worst-case tail L2 ~7e-4 at BAND=24), so applying M is a *banded* circulant
matmul.  The device kernel does:
  1. nonlinear phase rotation (intensity, mean over D, sin/cos on ACT)
  2. out = rot @ M as fp16 banded matmuls on the PE (PSUM-accumulated)
  3. PSUM evacuated as fp16 (ACT/Pool copies), stored un-rotated; the host
     applies the band's column roll and reassembles the complex output.
Data-parallel over B*S rows across the 8 cores; psi is pre-transposed on the
host so the contraction axis D sits on SBUF partitions (no device transposes).
"""

import math

import numpy as np

import concourse.bass as bass
import concourse.bacc as bacc
import concourse.mybir as mybir
from concourse.bass_utils import run_bass_kernel_spmd
from concourse.tile import TileContext

B, S, D = 8, 2048, 1024
N_CORES = 8
ROWS = B * S // N_CORES          # rows (B*S systems) per core = 2048
RC = 256                         # row-chunk size (pipeline unit)
N_RC = ROWS // RC                # 8
N_DC = D // 128                  # 8 d-blocks of 128 partitions
NUM_SCALES, SPARSITY = 3, 5
HALF_DT = 0.05
BAND = 24                        # circulant band half-width kept
WB = 128 + 2 * BAND              # matmul parallelogram width per d-block
F32 = mybir.dt.float32
F16 = mybir.dt.float16
AF = mybir.ActivationFunctionType
ALU = mybir.AluOpType

_cache = {}


def _mm_pieces(dc):
    """Banded MM for d-block dc writes psum cols q in [dc*128, dc*128+WB)
    (mod 1024; psum col q holds output index k=(q-BAND) mod 1024).  Split at
    the 1024-wrap and the 512-f32 PSUM bank boundary.
    Returns list of (bank, col_in_bank, j0, width); j indexes the WB-wide rhs."""
    q0 = (dc * 128) % 1024
    pieces = []
    j = 0
    while j < WB:
        q = (q0 + j) % 1024
        lim = WB - j
        lim = min(lim, 1024 - q)          # wrap split
        lim = min(lim, 512 - (q % 512))   # bank split
        pieces.append((q // 512, q % 512, j, lim))
        j += lim
    return pieces


def _build_program(uniform_alpha):
    nc = bacc.Bacc()
    psi_rt = nc.dram_tensor("psi_rt", [D, ROWS], F16, kind="ExternalInput")
    psi_it = nc.dram_tensor("psi_it", [D, ROWS], F16, kind="ExternalInput")
    mband = nc.dram_tensor("mband", [128, 3 * WB], F16, kind="ExternalInput")
    alpha_in = nc.dram_tensor("alpha", [D], F32, kind="ExternalInput")
    out_r = nc.dram_tensor("out_r", [ROWS, D], F16, kind="ExternalOutput")
    out_i = nc.dram_tensor("out_i", [ROWS, D], F16, kind="ExternalOutput")

    with TileContext(nc) as tc:
        with (
            tc.tile_pool(name="const", bufs=1) as constp,
            tc.tile_pool(name="work", bufs=3) as workp,
            tc.tile_pool(name="rot", bufs=3) as rotp,
            tc.tile_pool(name="small", bufs=4) as smallp,
            tc.tile_pool(name="outb", bufs=4) as outbp,
            tc.tile_pool(name="ps", bufs=3, space="PSUM") as psp,
            tc.tile_pool(name="psred", bufs=2, space="PSUM") as psredp,
        ):
            alpha_sb = constp.tile([128, N_DC], F32)
            nc.sync.dma_start(
                out=alpha_sb, in_=alpha_in.rearrange("(dc p) -> p dc", p=128)
            )
            ones_sq = constp.tile([128, 128], F16)
            nc.vector.memset(ones_sq, 1.0)
            halfpi = constp.tile([128, 1], F32)
            nc.vector.memset(halfpi, math.pi / 2.0)
            zerob = constp.tile([128, 1], F32)
            nc.vector.memset(zerob, 0.0)
            # fold alpha (uniform) and the 1/D mean scale into the reciprocal:
            # minv = 1/(rowsum*sc + bi); uniform: sc=1/(aD), bi=1e-8/a so the
            # Sin input is a*I/mean directly; else sc=1/D, bi=1e-8 and the Sin
            # applies per-partition alpha.
            sc_t = constp.tile([128, 1], F32)
            bi_t = constp.tile([128, 1], F32)
            if uniform_alpha:
                ralpha = constp.tile([128, 1], F32)
                nc.vector.reciprocal(ralpha, alpha_sb[:, 0:1])
                nc.vector.tensor_scalar_mul(sc_t, ralpha, 1.0 / float(D))
                nc.vector.tensor_scalar_mul(bi_t, ralpha, 1e-8)
            else:
                nc.vector.memset(sc_t, 1.0 / float(D))
                nc.vector.memset(bi_t, 1e-8)

            # whole-tensor fp16 loads (host pre-casts), SBUF free = (dc, r)
            # first chunk's rows load first so rc0 compute starts early;
            # mband is not needed until the first mm_stage, so load it after
            # the first psi pieces
            pr16 = constp.tile([128, N_DC * ROWS], F16)
            pi16 = constp.tile([128, N_DC * ROWS], F16)
            mband_sb = constp.tile([128, 3 * WB], F16)
            row_splits = [(0, RC), (RC, 2 * RC), (2 * RC, ROWS)]
            for si, (a, b) in enumerate(row_splits):
                if si == 1:
                    nc.sync.dma_start(out=mband_sb, in_=mband[:, :])
                for dst, src in ((pr16, psi_rt), (pi16, psi_it)):
                    src_ap = src[:, :]
                    dst3 = dst.rearrange("p (dc r) -> p dc r", dc=N_DC)
                    nc.sync.dma_start(
                        out=dst3[:, :, a:b],
                        in_=bass.AP(
                            tensor=src_ap.tensor,
                            offset=src_ap.offset + a,
                            ap=[[ROWS, 128], [128 * ROWS, N_DC], [1, b - a]],
                        ),
                    )

            def chunk_view(tile, r0, rcw):
                # [128, (dc, RC)] strided view of a [128, (dc, ROWS)] tile,
                # restricted to rows [r0, r0+RC); innermost stride stays 1.
                ap = tile[:, :]
                return bass.AP(
                    tensor=ap.tensor,
                    offset=ap.offset + r0,
                    ap=[list(ap.ap[0]), [ROWS, N_DC], [1, rcw]],
                )

            chunks = [(i * RC, (i + 1) * RC) for i in range(N_RC - 1)]
            chunks += [(ROWS - RC, ROWS - RC // 2), (ROWS - RC // 2, ROWS)]

            def _apslice(apv, rcw, c0, c1):
                # columns [c0, c1) of a [128, (dc, rcw)] chunk view, where
                # c0/c1 are multiples of rcw (whole dc-blocks)
                d0, d1 = c0 // rcw, c1 // rcw
                return bass.AP(
                    tensor=apv.tensor,
                    offset=apv.offset + d0 * apv.ap[1][0],
                    ap=[list(apv.ap[0]), [apv.ap[1][0], d1 - d0], [1, rcw]],
                )

            def phase_a(rc, r0, r1):
                # squares + ssum + PE rowsum: emitted BEFORE the previous
                # chunk's band matmuls so the rowsum (which gates this chunk's
                # whole phase chain) is not queued behind them on the in-order
                # PE
                RCW = r1 - r0
                W = N_DC * RCW
                prc = chunk_view(pr16, r0, RCW)
                pic = chunk_view(pi16, r0, RCW)
                sq_r = workp.tile([128, W], F16, tag="sq_r")
                sq_i = workp.tile([128, W], F16, tag="sq_i")
                nc.scalar.activation(sq_r, prc, AF.Square)
                nc.vector.tensor_mul(sq_i, pic, pic)
                ssum = workp.tile([128, W], F16, tag="ssum")
                nc.vector.tensor_add(ssum, sq_r, sq_i)

                # per-row sum over D via PE ones-reduction; ones is [128,128]
                # so every psum partition receives the sum (no broadcast op)
                ps_red = psredp.tile([128, RCW], F32, tag="psred")
                for dc in range(N_DC):
                    nc.tensor.matmul(
                        ps_red,
                        ones_sq,
                        ssum[:, dc * RCW : (dc + 1) * RCW],
                        start=(dc == 0),
                        stop=(dc == N_DC - 1),
                    )
                denom = smallp.tile([128, RCW], F32, tag="denom")
                nc.vector.tensor_scalar(
                    denom, ps_red, sc_t[:, 0:1], bi_t[:, 0:1],
                    op0=ALU.mult, op1=ALU.add,
                )
                minv_bc = smallp.tile([128, RCW], F16, tag="minvbc")
                with nc.allow_low_precision(reason="minv feeds fp16 phase anyway"):
                    nc.vector.reciprocal(minv_bc, denom)
                return ssum, minv_bc

            def phase_b(rc, r0, r1, ssum, minv_bc):
                RCW = r1 - r0
                W = N_DC * RCW
                prc = chunk_view(pr16, r0, RCW)
                pic = chunk_view(pi16, r0, RCW)

                # phase = alpha * intensity * minv (alpha folded into minv when
                # uniform); c = cos, s = sin via ACT
                phs = workp.tile([128, W], F16, tag="phs")
                mb_ap = minv_bc[:, 0:RCW]
                minv_rep = bass.AP(
                    tensor=mb_ap.tensor,
                    offset=mb_ap.offset,
                    ap=[list(mb_ap.ap[0]), [0, N_DC], [1, RCW]],
                )
                nc.vector.tensor_mul(phs, ssum, minv_rep)
                cc = rotp.tile([128, W], F16, tag="cc")
                ss = rotp.tile([128, W], F16, tag="ss")
                if uniform_alpha:
                    nc.scalar.activation(cc, phs, AF.Sin, bias=halfpi[:, 0:1])
                    nc.scalar.activation(ss, phs, AF.Sin, bias=zerob[:, 0:1])
                else:
                    for dc in range(N_DC):
                        sl = slice(dc * RCW, (dc + 1) * RCW)
                        nc.scalar.activation(
                            cc[:, sl], phs[:, sl], AF.Sin,
                            bias=halfpi[:, 0:1], scale=alpha_sb[:, dc : dc + 1],
                        )
                        nc.scalar.activation(
                            ss[:, sl], phs[:, sl], AF.Sin,
                            bias=zerob[:, 0:1], scale=alpha_sb[:, dc : dc + 1],
                        )
                # rotation: xr = pr*c - pi*s ; xi = pr*s + pi*c   (fp16)
                t1 = rotp.tile([128, W], F16, tag="ta")
                t2 = rotp.tile([128, W], F16, tag="tb")
                t3 = rotp.tile([128, W], F16, tag="ta", name=f"t3_{rc}")
                t4 = rotp.tile([128, W], F16, tag="tb", name=f"t4_{rc}")
                xr = rotp.tile([128, W], F16, tag="xr")
                xi = rotp.tile([128, W], F16, tag="xi")
                nc.gpsimd.tensor_mul(t1, cc, prc)
                nc.vector.tensor_mul(t2, pic, ss)
                nc.vector.tensor_sub(xr, t1, t2)
                nc.vector.tensor_mul(t3, prc, ss)
                hsp = (5 * N_DC // 8) * RCW
                nc.vector.tensor_mul(
                    t4[:, 0:hsp], _apslice(pic, RCW, 0, hsp), cc[:, 0:hsp]
                )
                nc.gpsimd.tensor_mul(
                    t4[:, hsp:W], _apslice(pic, RCW, hsp, W), cc[:, hsp:W]
                )
                nc.vector.tensor_add(xi, t3, t4)

                return xr, xi

            def mm_stage(rc, r0, r1, xr, xi):
                RCW = r1 - r0
                # banded circulant matmul, row-blocks of 128
                for rbl in range(RCW // 128):
                    pst = {
                        comp: psp.tile(
                            [128, D], F32, tag="ps", name=f"ps_{comp}_{rc}_{rbl}"
                        )
                        for comp in ("r", "i")
                    }
                    # xr-groups first: xi (slowest producer) can lag without
                    # stalling the PE
                    plan = []  # ((comp, bank), psum_col, width, lhsT_ap, rhs_ap)
                    for xt, mat, comp in (
                        (xr, 0, "r"), (xr, 1, "i"), (xi, 0, "i"), (xi, 2, "r"),
                    ):
                        for dc in range(N_DC):
                            c0 = dc * RCW + rbl * 128
                            lhsT = xt[:, c0 : c0 + 128]
                            for bank, col, j0, wdt in _mm_pieces(dc):
                                rhs = mband_sb[:, mat * WB + j0 : mat * WB + j0 + wdt]
                                plan.append(
                                    ((comp, bank), bank * 512 + col, wdt, lhsT, rhs)
                                )
                    first, last = {}, {}
                    for idx, (key, *_rest) in enumerate(plan):
                        first.setdefault(key, idx)
                        last[key] = idx
                    for idx, (key, col, wdt, lhsT, rhs) in enumerate(plan):
                        nc.tensor.matmul(
                            pst[key[0]][:, col : col + wdt],
                            lhsT,
                            rhs,
                            start=(first[key] == idx),
                            stop=(last[key] == idx),
                            skip_group_check=True,
                        )

                    # evacuate psum as fp16 (cast): gpsimd cannot read PSUM, so
                    # split between ACT and DVE; host applies the -BAND roll
                    rb = r0 // 128 + rbl
                    for comp, dram in (("i", out_i), ("r", out_r)):
                        ev = outbp.tile([128, D], F16, tag=f"ev{comp}")
                        if rb >= 14 and comp == "r":
                            # drain: DVE is idle by now — evacuate r there so
                            # the last two row-blocks' copies run in parallel
                            nc.vector.tensor_copy(ev[:, :], pst[comp][:, :])
                        else:
                            nc.scalar.copy(ev[:, :], pst[comp][:, :])
                        nc.sync.dma_start(
                            out=dram[rb * 128 : (rb + 1) * 128, :], in_=ev[:, :]
                        )

            # baseline-style software pipeline: phase(c) emitted before
            # mm(c-1) so the next chunk's elementwise work queues ahead of the
            # previous chunk's matmuls on each engine
            pending = None
            for rc, (r0, r1) in enumerate(chunks):
                sred = phase_a(rc, r0, r1)
                xrxi = phase_b(rc, r0, r1, *sred)
                if pending is not None:
                    mm_stage(*pending)
                pending = (rc, r0, r1, *xrxi)
            mm_stage(*pending)
    return nc


def _host_mband(ham_w):
    """Band tile of the Cayley circulant: entry [p, j] = M[d, k] at relative
    offset k-d = j-BAND-p (shift-invariant across d-blocks)."""
    k = np.arange(D)
    lam = np.zeros(D, dtype=np.float64)
    w = np.asarray(ham_w, dtype=np.float64)
    for m in range(NUM_SCALES):
        for j in range(SPARSITY):
            off = (2 ** m) * (j + 1)
            lam += w[m, j] * 2.0 * (1.0 - np.cos(2.0 * np.pi * off * k / D))
    g = (1.0 - 1j * HALF_DT * lam) / (1.0 + 1j * HALF_DT * lam)
    ccol = np.fft.ifft(g)
    rel = (np.arange(WB)[None, :] - BAND - np.arange(128)[:, None]) % D
    Mr = ccol.real[rel]
    Mi = ccol.imag[rel]
    return np.concatenate([Mr, Mi, -Mi], axis=1).astype(np.float16)


def kernel(psi_r, psi_i, alpha, ham_w):
    psi_r = np.asarray(psi_r, dtype=np.float32)
    psi_i = np.asarray(psi_i, dtype=np.float32)
    alpha = np.asarray(alpha, dtype=np.float32)

    uniform = bool(np.all(alpha == alpha.flat[0]))
    key = ("nc", uniform)
    if key not in _cache:
        nc = _build_program(uniform)
        nc.finalize()
        _cache[key] = nc
    nc = _cache[key]

    mband = _host_mband(ham_w)
    prT = np.ascontiguousarray(psi_r.reshape(B * S, D).T.astype(np.float16))
    piT = np.ascontiguousarray(psi_i.reshape(B * S, D).T.astype(np.float16))

    in_maps = []
    for c in range(N_CORES):
        sl = slice(c * ROWS, (c + 1) * ROWS)
        in_maps.append(
            {
                "psi_rt": np.ascontiguousarray(prT[:, sl]),
                "psi_it": np.ascontiguousarray(piT[:, sl]),
                "mband": mband,
                "alpha": alpha,
            }
        )
    res = run_bass_kernel_spmd(nc, in_maps, core_ids=list(range(N_CORES)))
    _cache["last_run"] = res
    outr = np.concatenate([r["out_r"] for r in res.results], axis=0)
    outi = np.concatenate([r["out_i"] for r in res.results], axis=0)
    # psum col q holds output index k = (q - BAND) mod D -> roll left by BAND
    outr = np.roll(outr.astype(np.float32), -BAND, axis=-1)
    outi = np.roll(outi.astype(np.float32), -BAND, axis=-1)
    full = np.stack([outr, outi], axis=-1)
    return full.reshape(B, S, D, 2)
